# revision 1
# baseline (speedup 1.0000x reference)
"""Minkowski sparse conv-transpose kernel for 8 trn2 NeuronCores.

Sharding: pairs (k, m) are assigned to cores by output-row range
(out_map // 50000 -> core). Each core gathers feats rows for its pairs
(SWDGE indirect DMA, 128 rows/instruction), applies the per-offset 32x32
weight on TensorE (DVE 32x32 block-transpose puts channels on partitions;
4 concurrent 32x32 array tiles via tile_position), and scatter-accumulates
into its private 50000-row output slice via SWDGE indirect DMA with CCE
add. No collectives; the host concatenates the 8 slices.

Scatter correctness: descriptors within one indirect DMA instruction run
concurrently on 16 SDMA engines, so every 128-pair chunk is built on the
host to contain distinct output rows (pairs split by per-(core,k)
occurrence rank, each rank padded to a 128 multiple; padding scatters to
per-lane trash rows 50000+lane). Distinct chunks are serialized by Tile's
DRAM WAW tracking.
"""
import numpy as np

import concourse.bass as bass
import concourse.mybir as mybir
import concourse.tile as tile
from concourse.bass_utils import run_bass_kernel_spmd

dt = mybir.dt

NCORES = 8
K = 27
N_IN = 200000
N_OUT = 400000
C = 32
ROWS_PER_CORE = N_OUT // NCORES  # 50000
ACC_ROWS = 50176  # 50000 real + trash rows, 128-multiple
LANES = 128
SLOTS = 16  # chunks per burst => [128, 512] tiles


def _split_dma_waits(nc, max_waits=1):
    """This toolchain allows only one sync wait per instruction; hoist
    extras onto a chain of single-wait NoOps ahead of the instruction."""
    for bb in nc.main_func.blocks:
        out = []
        for ins in bb.instructions:
            if ins.sync_info is not None and len(ins.sync_info.on_wait) > max_waits:
                waits = list(ins.sync_info.on_wait)
                extra, keep = waits[:-max_waits], waits[-max_waits:]
                for i, w in enumerate(extra):
                    nop = mybir.InstNoOp(name=f"{ins.name}-ws{i}", ins=[], outs=[])
                    nop.engine = ins.engine
                    nop.sync_info = mybir.SyncInfo(on_wait=[w], on_update=[])
                    out.append(nop)
                ins.sync_info = mybir.SyncInfo(
                    on_wait=keep, on_update=list(ins.sync_info.on_update)
                )
            out.append(ins)
        bb.instructions[:] = out


def _trash_rows(n):
    return (50000 + (np.arange(n) % 176)).astype(np.int64)


def _core_chunks(in_map, out_map, core):
    """[nch,128] gather-row / local-scatter-row arrays per k, chunk-unique."""
    lo = core * ROWS_PER_CORE
    hi = lo + ROWS_PER_CORE
    gk_list, sk_list, r0_list = [], [], []
    for k in range(K):
        om = out_map[k]
        sel = (om >= lo) & (om < hi)
        gi = in_map[k][sel].astype(np.int64)
        lr = (om[sel] - lo).astype(np.int64)
        order = np.argsort(lr, kind="stable")
        lr_s, gi_s = lr[order], gi[order]
        n = lr_s.size
        if n == 0:
            gk_list.append(np.zeros((1, LANES), np.int64))
            sk_list.append(_trash_rows(LANES).reshape(1, LANES))
            r0_list.append(1)
            continue
        is_new = np.ones(n, dtype=bool)
        is_new[1:] = lr_s[1:] != lr_s[:-1]
        seg_start = np.where(is_new)[0]
        seg_id = np.cumsum(is_new) - 1
        rank = np.arange(n) - seg_start[seg_id]
        parts_g, parts_s = [], []
        r0_chunks = 0
        for r in range(int(rank.max()) + 1):
            m = rank == r
            gr, sr = gi_s[m], lr_s[m]
            pad = (-gr.size) % LANES
            gr = np.concatenate([gr, np.zeros(pad, np.int64)])
            sr = np.concatenate([sr, _trash_rows(pad)])
            parts_g.append(gr)
            parts_s.append(sr)
            if r == 0:
                r0_chunks = gr.size // LANES
        gk = np.concatenate(parts_g).reshape(-1, LANES)
        sk = np.concatenate(parts_s).reshape(-1, LANES)
        gk_list.append(gk)
        sk_list.append(sk)
        r0_list.append(r0_chunks)
    return gk_list, sk_list, r0_list


_CACHE = {}
_LAST_IN_MAPS = None


def _build_program(nb_per_k, r0_per_k=None):
    key = (tuple(nb_per_k), tuple(r0_per_k) if r0_per_k else None)
    if key in _CACHE:
        return _CACHE[key]
    nb_total = sum(nb_per_k)

    nc = bass.Bass()
    feats = nc.declare_dram_parameter("feats", [N_IN, C], dt.float32, isOutput=False)
    wstack = nc.declare_dram_parameter(
        "wstack", [K, 128, C], dt.float32, isOutput=False
    )
    gidx = nc.declare_dram_parameter(
        "gidx", [nb_total, LANES, SLOTS], dt.int32, isOutput=False
    )
    sidx = nc.declare_dram_parameter(
        "sidx", [nb_total, LANES, SLOTS], dt.int32, isOutput=False
    )
    acc = nc.declare_dram_parameter("acc", [ACC_ROWS, C], dt.float32, isOutput=True)

    with tile.TileContext(nc) as tc:
        with (
            tc.tile_pool(name="sbuf", bufs=3) as sb,
            tc.tile_pool(name="wpool", bufs=2) as wp,
            tc.tile_pool(name="zpool", bufs=1) as zp,
            tc.tile_pool(name="psum", bufs=2, space="PSUM") as ps,
        ):
            zero_t = zp.tile([128, 1024], dt.float32)
            nc.gpsimd.memset(zero_t[:], 0.0)
            r0 = 0
            while r0 < ACC_ROWS:
                r1 = min(r0 + 4096, ACC_ROWS)
                cols = (r1 - r0) * C // 128
                nc.sync.dma_start(out=acc[r0:r1, :], in_=zero_t[:, :cols])
                r0 = r1
            tc.strict_bb_all_engine_barrier()

            fake_ctr = [0]

            def scatter(si_ap, v_ap, fake):
                out_ap = acc[0:128, :]
                if fake:
                    fake_ctr[0] += 1
                    out_ap = bass.AP(
                        tensor=out_ap.tensor,
                        offset=out_ap.offset,
                        ap=out_ap.ap,
                        dep_tracking_offset=fake_ctr[0] * ACC_ROWS * C,
                    )
                nc.gpsimd.indirect_dma_start(
                    out=out_ap,
                    out_offset=bass.IndirectOffsetOnAxis(ap=si_ap, axis=0),
                    in_=v_ap,
                    in_offset=None,
                    compute_op=mybir.AluOpType.add,
                )

            b_base = 0
            for k in range(K):
                w4 = wp.tile([128, C], dt.float32, tag="w4", name="w4")
                nc.sync.dma_start(out=w4[:], in_=wstack[k])
                for b in range(nb_per_k[k]):
                    gi = sb.tile([128, SLOTS], dt.int32, tag="gi", name="gi")
                    si = sb.tile([128, SLOTS], dt.int32, tag="si", name="si")
                    nc.sync.dma_start(out=gi[:], in_=gidx[b_base + b])
                    nc.sync.dma_start(out=si[:], in_=sidx[b_base + b])
                    x = sb.tile([128, SLOTS, C], dt.float32, tag="x", name="x")
                    for j in range(SLOTS):
                        nc.gpsimd.indirect_dma_start(
                            out=x[:, j, :],
                            out_offset=None,
                            in_=feats[0:128, :],
                            in_offset=bass.IndirectOffsetOnAxis(
                                ap=gi[:, j : j + 1], axis=0
                            ),
                        )
                    xt = sb.tile([128, SLOTS * C], dt.float32, tag="xt", name="xt")
                    nc.vector.transpose(out=xt[:], in_=x[:])
                    pt = ps.tile([128, SLOTS * C], dt.float32, tag="pt", name="pt")
                    for r in range(4):
                        nc.tensor.matmul(
                            out=pt[32 * r : 32 * r + 32, :],
                            lhsT=w4[32 * r : 32 * r + 32, :],
                            rhs=xt[32 * r : 32 * r + 32, :],
                            start=True,
                            stop=True,
                            tile_position=(32 * r, 32 * r),
                        )
                    v = sb.tile([128, SLOTS, C], dt.float32, tag="v", name="v")
                    nc.vector.transpose(out=v[:], in_=pt[:])
                    for j in range(SLOTS):
                        chunk_id = b * SLOTS + j
                        fake = r0_per_k is not None and chunk_id < r0_per_k[k]
                        if (
                            r0_per_k is not None
                            and chunk_id == r0_per_k[k]
                        ):
                            tc.strict_bb_all_engine_barrier()
                        scatter(si[:, j : j + 1], v[:, j, :], fake)
                if r0_per_k is not None:
                    tc.strict_bb_all_engine_barrier()
                b_base += nb_per_k[k]
    _split_dma_waits(nc)
    _CACHE[key] = nc
    return nc


def kernel(feats, kernel, in_map, out_map, n_out):
    feats = np.ascontiguousarray(np.asarray(feats, dtype=np.float32))
    wk = np.asarray(kernel, dtype=np.float32)
    in_map = np.asarray(in_map, dtype=np.int32)
    out_map = np.asarray(out_map, dtype=np.int32)

    import os

    per_core = [_core_chunks(in_map, out_map, c) for c in range(NCORES)]
    # unified burst counts per k (max across cores)
    nb_per_k = []
    for k in range(K):
        nch = max(per_core[c][0][k].shape[0] for c in range(NCORES))
        nb_per_k.append((nch + SLOTS - 1) // SLOTS)
    if os.environ.get("KERNEL_FAST", "1") == "1":
        # rank-0 scatter chunks are mutually conflict-free within each k:
        # issue them with faked (disjoint) dep-tracking so they run
        # concurrently; barrier before the rank>=1 tail and after each k.
        r0_per_k = [
            min(per_core[c][2][k] for c in range(NCORES)) for k in range(K)
        ]
    else:
        r0_per_k = None
    nc = _build_program(nb_per_k, r0_per_k)

    wstack = np.tile(wk, (1, 4, 1)).reshape(K, 128, C).astype(np.float32)

    in_maps = []
    for c in range(NCORES):
        gk_list, sk_list, _r0 = per_core[c]
        gbursts, sbursts = [], []
        for k in range(K):
            tgt = nb_per_k[k] * SLOTS
            gk, sk = gk_list[k], sk_list[k]
            padc = tgt - gk.shape[0]
            if padc:
                gk = np.concatenate([gk, np.zeros((padc, LANES), np.int64)])
                sk = np.concatenate(
                    [sk, np.tile(_trash_rows(LANES), (padc, 1))]
                )
            # [nb, SLOTS, 128] -> [nb, 128, SLOTS]
            gbursts.append(gk.reshape(-1, SLOTS, LANES).transpose(0, 2, 1))
            sbursts.append(sk.reshape(-1, SLOTS, LANES).transpose(0, 2, 1))
        gidx = np.ascontiguousarray(
            np.concatenate(gbursts, axis=0).astype(np.int32)
        )
        sidx = np.ascontiguousarray(
            np.concatenate(sbursts, axis=0).astype(np.int32)
        )
        in_maps.append(dict(feats=feats, wstack=wstack, gidx=gidx, sidx=sidx))

    global _LAST_IN_MAPS
    _LAST_IN_MAPS = in_maps
    res = run_bass_kernel_spmd(nc, in_maps, list(range(NCORES)))
    out = np.concatenate(
        [res.results[c]["acc"][:ROWS_PER_CORE] for c in range(NCORES)], axis=0
    )
    return out.astype(np.float32)

